# revision 17
# baseline (speedup 1.0000x reference)
"""EntropyBottleneck forward (q_mode='noise') as a Trainium2 Bass kernel.

Math
----
reference computes, per channel c with tiny per-channel params (W_k, b_k, f_k):

    y    = x + noise
    v    = y flattened per channel
    L(v) = chain of FactorizeCell: u <- softplus(W_k) @ u + b_k,
           then u <- u + tanh(f_k) * tanh(u)   (for k < last)
    lower = L(v - 0.5); upper = L(v + 0.5)
    s     = -sign(lower + upper)
    lik   = max(|sigmoid(s*upper) - sigmoid(s*lower)|, 1e-9)

When every gate f_k == 0 (true for this module's initialization), the chain is
per-channel *affine*: L(v) = M_c * v + D_c, with M_c > 0 (product of softplus
matrices) and D_c foldable on the host from the (C,3,3)-at-most params.
Then with h = M_c/2:

    lower = t - h,  upper = t + h,  where t = M_c * y + D_c
    lik   = |sigmoid(s*(t+h)) - sigmoid(s*(t-h))|
          = sigmoid(h - |t|) - sigmoid(-h - |t|)           (sign trick folded)
          = 0.5 * (tanh((t + h)/2) - tanh((t - h)/2))      (tanh identity,
                                                            sign-free: always >= 0)

The device kernel therefore does, per element:
    y   = x + noise                                  (vector engine)
    p   = tanh(M/2 * y + (D + h)/2)                  (scalar engine, fused affine)
    q   = tanh(M/2 * y + (D - h)/2)                  (scalar engine, fused affine)
    lik = max(0.5 * (p - q), 1e-9)                   (vector engine, fused)

Sharding: data-parallel over batch, one batch element per NeuronCore (8 cores).
Per-core tensor (192, 4096) is viewed as (384, 2048): row r holds half of
channel r//2, so each SBUF partition maps to exactly one channel and the
per-channel coefficients become per-partition scale/bias operands.
"""

import numpy as np

B, C, H, W = 8, 192, 64, 64
NCORES = 8
ROWS, COLS = 384, 2048  # (C, H*W) = (192, 4096) viewed as (384, 2048)
NT = ROWS // 128  # 3 row-tiles of 128 partitions

_CACHE: dict = {}


def _softplus64(x: np.ndarray) -> np.ndarray:
    x = x.astype(np.float64)
    return np.log1p(np.exp(-np.abs(x))) + np.maximum(x, 0.0)


def _fold_affine(ws, bs):
    """Compose the per-channel affine chain: L(v) = M*v + D. Returns (M, D) as (C,)."""
    M = np.ones((C, 1, 1), np.float64)
    D = np.zeros((C, 1, 1), np.float64)
    for Wk, bk in zip(ws, bs):
        spw = _softplus64(np.asarray(Wk))
        M = spw @ M
        D = spw @ D + np.asarray(bk, np.float64)
    return M[:, 0, 0], D[:, 0, 0]


def _numpy_fallback(x, noise, ws, bs, fs):
    """Exact replica of the reference chain for the general (gated) case."""
    x = np.asarray(x, np.float32)
    noise = np.asarray(noise, np.float32)
    y = x + noise
    v = y.transpose(1, 0, 2, 3).reshape(C, 1, -1).astype(np.float32)

    def logits(v):
        for i, (Wk, bk) in enumerate(zip(ws, bs)):
            spw = _softplus64(np.asarray(Wk)).astype(np.float32)
            v = np.einsum("coi,cin->con", spw, v) + np.asarray(bk, np.float32)
            if i < len(fs):
                v = v + np.tanh(np.asarray(fs[i], np.float32)) * np.tanh(v)
        return v

    lower = logits(v - 0.5)
    upper = logits(v + 0.5)
    sign = -np.sign(lower + upper)
    sig = lambda z: 1.0 / (1.0 + np.exp(-z, dtype=np.float32))
    lik = np.abs(sig(sign * upper) - sig(sign * lower))
    lik = np.maximum(lik, np.float32(1e-9))
    lik = lik.reshape(C, B, H, W).transpose(1, 0, 2, 3)
    return y, lik


def _build_program():
    import concourse.bacc as bacc
    import concourse.mybir as mybir
    import concourse.tile as tile

    f32 = mybir.dt.float32
    nc = bacc.Bacc("TRN2", target_bir_lowering=False, debug=False,
                   num_devices=NCORES)

    x_d = nc.dram_tensor("x", [ROWS, COLS], f32, kind="ExternalInput")
    n_d = nc.dram_tensor("noise", [ROWS, COLS], f32, kind="ExternalInput")
    sc_d = nc.dram_tensor("scl", [128, NT], f32, kind="ExternalInput")
    bp_d = nc.dram_tensor("bp", [128, NT], f32, kind="ExternalInput")
    bq_d = nc.dram_tensor("bq", [128, NT], f32, kind="ExternalInput")
    y_d = nc.dram_tensor("y", [ROWS, COLS], f32, kind="ExternalOutput")
    l_d = nc.dram_tensor("lik", [ROWS, COLS], f32, kind="ExternalOutput")

    Tanh = mybir.ActivationFunctionType.Tanh
    op_add = mybir.AluOpType.add
    op_sub = mybir.AluOpType.subtract
    op_mult = mybir.AluOpType.mult
    op_max = mybir.AluOpType.max

    with tile.TileContext(nc) as tc:
        with (
            tc.tile_pool(name="const", bufs=1) as cpool,
            tc.tile_pool(name="io", bufs=1) as iopool,
            tc.tile_pool(name="tmp", bufs=6) as tpool,
        ):
            sc = cpool.tile([128, NT], f32, tag="sc")
            nc.sync.dma_start(sc[:], sc_d[:])
            bp = cpool.tile([128, NT], f32, tag="bp")
            nc.sync.dma_start(bp[:], bp_d[:])
            bq = cpool.tile([128, NT], f32, tag="bq")
            nc.sync.dma_start(bq[:], bq_d[:])

            # Loads are split across the two HWDGE FIFOs (SP via nc.sync,
            # ACT via nc.scalar) and issue before any store enters either
            # FIFO, so nothing delays the load stream. Stores then drain
            # behind them: y tiles behind the sync loads, lik chunks behind
            # the scalar loads. The lik compute chain is chunked so the last
            # ring items are ready before the rings drain.
            CH = 1024
            NCH = COLS // CH
            xts, nts = [], []
            for t in range(NT):
                rows = slice(t * 128, (t + 1) * 128)
                xt = iopool.tile([128, COLS], f32, tag=f"xt{t}")
                nt = iopool.tile([128, COLS], f32, tag=f"nt{t}")
                if t == 0:
                    # Tile 0's loads are split so the first add (and with it
                    # the whole scalar stream) starts ~4us earlier.
                    half = COLS // 2
                    nc.sync.dma_start(xt[:, :half], x_d[rows, :half])
                    nc.sync.dma_start(nt[:, :half], n_d[rows, :half])
                    nc.sync.dma_start(xt[:, half:], x_d[rows, half:])
                    nc.sync.dma_start(nt[:, half:], n_d[rows, half:])
                else:
                    nc.sync.dma_start(xt[:], x_d[rows, :])
                    nc.sync.dma_start(nt[:], n_d[rows, :])
                xts.append(xt)
                nts.append(nt)

            yts = []
            for t in range(NT):
                rows = slice(t * 128, (t + 1) * 128)
                yt = iopool.tile([128, COLS], f32, tag=f"yt{t}")
                if t == 0:
                    half = COLS // 2
                    nc.vector.tensor_tensor(yt[:, :half], xts[t][:, :half],
                                            nts[t][:, :half], op=op_add)
                    nc.vector.tensor_tensor(yt[:, half:], xts[t][:, half:],
                                            nts[t][:, half:], op=op_add)
                else:
                    nc.vector.tensor_tensor(yt[:], xts[t][:], nts[t][:],
                                            op=op_add)
                yts.append(yt)
                # Finer chunks on the last tile keep its compute+store tail
                # short; the rings are already saturated for earlier tiles.
                ch = CH // 2 if t == NT - 1 else CH
                for c in range(COLS // ch):
                    cols = slice(c * ch, (c + 1) * ch)
                    pt = tpool.tile([128, ch], f32, tag=f"pt{t % 2}")
                    nc.scalar.activation(pt[:], yt[:, cols], Tanh,
                                         bias=bp[:, t:t + 1], scale=sc[:, t:t + 1])
                    qt = tpool.tile([128, ch], f32, tag=f"qt{t % 2}")
                    nc.scalar.activation(qt[:], yt[:, cols], Tanh,
                                         bias=bq[:, t:t + 1], scale=sc[:, t:t + 1])

                    nc.vector.tensor_tensor(pt[:], pt[:], qt[:], op=op_sub)
                    nc.vector.tensor_scalar(pt[:], pt[:], 0.5, 1e-9,
                                            op0=op_mult, op1=op_max)
                    nc.scalar.dma_start(l_d[rows, cols], pt[:])

            # y stores ride the sync FIFO BEHIND its loads: by the time the
            # FIFO drains the loads, every y tile is already computed, so the
            # final ring work is never compute-gated.
            for t in range(NT):
                rows = slice(t * 128, (t + 1) * 128)
                nc.sync.dma_start(y_d[rows, :], yts[t][:])

    nc.compile()
    return nc


def _get_program():
    if "nc" not in _CACHE:
        _CACHE["nc"] = _build_program()
    return _CACHE["nc"]


def kernel(x, noise, w0, b0, f0, w1, b1, f1, w2, b2, f2, w3, b3):
    from concourse.bass_utils import run_bass_kernel_spmd

    ws = [w0, w1, w2, w3]
    bs = [b0, b1, b2, b3]
    fs = [f0, f1, f2]

    if any(np.any(np.asarray(f) != 0.0) for f in fs):
        # Gated (non-affine) case: bit-accurate host fallback. Never taken for
        # this module's initialization (all gates are zero).
        return _numpy_fallback(x, noise, ws, bs, fs)

    M, D = _fold_affine(ws, bs)  # (C,) float64 each, M > 0
    ch = np.arange(ROWS) // 2  # channel id per folded row
    Mr, Dr = M[ch], D[ch]
    # p/q = tanh(M/2 * y + (D +- M/2)/2); lik = max(0.5*(p - q), 1e-9)
    scl = (Mr / 2).astype(np.float32).reshape(NT, 128).T.copy()
    bpv = (Dr / 2 + Mr / 4).astype(np.float32).reshape(NT, 128).T.copy()
    bqv = (Dr / 2 - Mr / 4).astype(np.float32).reshape(NT, 128).T.copy()

    x = np.ascontiguousarray(np.asarray(x, np.float32))
    noise = np.ascontiguousarray(np.asarray(noise, np.float32))

    nc = _get_program()
    in_maps = [
        {
            "x": x[b].reshape(ROWS, COLS),
            "noise": noise[b].reshape(ROWS, COLS),
            "scl": scl,
            "bp": bpv,
            "bq": bqv,
        }
        for b in range(NCORES)
    ]
    res = run_bass_kernel_spmd(nc, in_maps, list(range(NCORES))).results

    y = np.stack([res[b]["y"].reshape(C, H, W) for b in range(NCORES)])
    lik = np.stack([res[b]["lik"].reshape(C, H, W) for b in range(NCORES)])
    return y, lik


# revision 18
# speedup vs baseline: 1.1140x; 1.1140x over previous
"""EntropyBottleneck forward (q_mode='noise') as a Trainium2 Bass kernel.

Math
----
reference computes, per channel c with tiny per-channel params (W_k, b_k, f_k):

    y    = x + noise
    v    = y flattened per channel
    L(v) = chain of FactorizeCell: u <- softplus(W_k) @ u + b_k,
           then u <- u + tanh(f_k) * tanh(u)   (for k < last)
    lower = L(v - 0.5); upper = L(v + 0.5)
    s     = -sign(lower + upper)
    lik   = max(|sigmoid(s*upper) - sigmoid(s*lower)|, 1e-9)

When every gate f_k == 0 (true for this module's initialization), the chain is
per-channel *affine*: L(v) = M_c * v + D_c, with M_c > 0 (product of softplus
matrices) and D_c foldable on the host from the (C,3,3)-at-most params.
Then with h = M_c/2:

    lower = t - h,  upper = t + h,  where t = M_c * y + D_c
    lik   = |sigmoid(s*(t+h)) - sigmoid(s*(t-h))|
          = sigmoid(h - |t|) - sigmoid(-h - |t|)           (sign trick folded)
          = 0.5 * (tanh((t + h)/2) - tanh((t - h)/2))      (tanh identity,
                                                            sign-free: always >= 0)

The device kernel therefore does, per element:
    y   = x + noise                                  (vector engine)
    p   = tanh(M/2 * y + (D + h)/2)                  (scalar engine, fused affine)
    q   = tanh(M/2 * y + (D - h)/2)                  (scalar engine, fused affine)
    lik = max(0.5 * (p - q), 1e-9)                   (vector engine, fused)

Sharding: data-parallel over batch, one batch element per NeuronCore (8 cores).
Per-core tensor (192, 4096) is viewed as (384, 2048): row r holds half of
channel r//2, so each SBUF partition maps to exactly one channel and the
per-channel coefficients become per-partition scale/bias operands.
"""

import numpy as np

B, C, H, W = 8, 192, 64, 64
NCORES = 8
ROWS, COLS = 384, 2048  # (C, H*W) = (192, 4096) viewed as (384, 2048)
NT = ROWS // 128  # 3 row-tiles of 128 partitions

_CACHE: dict = {}


def _softplus64(x: np.ndarray) -> np.ndarray:
    x = x.astype(np.float64)
    return np.log1p(np.exp(-np.abs(x))) + np.maximum(x, 0.0)


def _fold_affine(ws, bs):
    """Compose the per-channel affine chain: L(v) = M*v + D. Returns (M, D) as (C,)."""
    M = np.ones((C, 1, 1), np.float64)
    D = np.zeros((C, 1, 1), np.float64)
    for Wk, bk in zip(ws, bs):
        spw = _softplus64(np.asarray(Wk))
        M = spw @ M
        D = spw @ D + np.asarray(bk, np.float64)
    return M[:, 0, 0], D[:, 0, 0]


def _numpy_fallback(x, noise, ws, bs, fs):
    """Exact replica of the reference chain for the general (gated) case."""
    x = np.asarray(x, np.float32)
    noise = np.asarray(noise, np.float32)
    y = x + noise
    v = y.transpose(1, 0, 2, 3).reshape(C, 1, -1).astype(np.float32)

    def logits(v):
        for i, (Wk, bk) in enumerate(zip(ws, bs)):
            spw = _softplus64(np.asarray(Wk)).astype(np.float32)
            v = np.einsum("coi,cin->con", spw, v) + np.asarray(bk, np.float32)
            if i < len(fs):
                v = v + np.tanh(np.asarray(fs[i], np.float32)) * np.tanh(v)
        return v

    lower = logits(v - 0.5)
    upper = logits(v + 0.5)
    sign = -np.sign(lower + upper)
    sig = lambda z: 1.0 / (1.0 + np.exp(-z, dtype=np.float32))
    lik = np.abs(sig(sign * upper) - sig(sign * lower))
    lik = np.maximum(lik, np.float32(1e-9))
    lik = lik.reshape(C, B, H, W).transpose(1, 0, 2, 3)
    return y, lik


def _build_program():
    import concourse.bacc as bacc
    import concourse.mybir as mybir
    import concourse.tile as tile

    f32 = mybir.dt.float32
    nc = bacc.Bacc("TRN2", target_bir_lowering=False, debug=False,
                   num_devices=NCORES)

    x_d = nc.dram_tensor("x", [ROWS, COLS], f32, kind="ExternalInput")
    n_d = nc.dram_tensor("noise", [ROWS, COLS], f32, kind="ExternalInput")
    sc_d = nc.dram_tensor("scl", [128, NT], f32, kind="ExternalInput")
    bp_d = nc.dram_tensor("bp", [128, NT], f32, kind="ExternalInput")
    bq_d = nc.dram_tensor("bq", [128, NT], f32, kind="ExternalInput")
    y_d = nc.dram_tensor("y", [ROWS, COLS], f32, kind="ExternalOutput")
    l_d = nc.dram_tensor("lik", [ROWS, COLS], f32, kind="ExternalOutput")

    Tanh = mybir.ActivationFunctionType.Tanh
    op_add = mybir.AluOpType.add
    op_sub = mybir.AluOpType.subtract
    op_mult = mybir.AluOpType.mult
    op_max = mybir.AluOpType.max

    with tile.TileContext(nc) as tc:
        with (
            tc.tile_pool(name="const", bufs=1) as cpool,
            tc.tile_pool(name="io", bufs=1) as iopool,
            tc.tile_pool(name="tmp", bufs=6) as tpool,
        ):
            sc = cpool.tile([128, NT], f32, tag="sc")
            nc.sync.dma_start(sc[:], sc_d[:])
            bp = cpool.tile([128, NT], f32, tag="bp")
            nc.sync.dma_start(bp[:], bp_d[:])
            bq = cpool.tile([128, NT], f32, tag="bq")
            nc.sync.dma_start(bq[:], bq_d[:])

            # Loads are split across the two HWDGE FIFOs (SP via nc.sync,
            # ACT via nc.scalar) and issue before any store enters either
            # FIFO, so nothing delays the load stream. Stores then drain
            # behind them: y tiles behind the sync loads, lik chunks behind
            # the scalar loads. The lik compute chain is chunked so the last
            # ring items are ready before the rings drain.
            CH = 1024
            NCH = COLS // CH
            xts, nts = [], []
            for t in range(NT):
                rows = slice(t * 128, (t + 1) * 128)
                xt = iopool.tile([128, COLS], f32, tag=f"xt{t}")
                nt = iopool.tile([128, COLS], f32, tag=f"nt{t}")
                if t == 0:
                    # Tile 0's loads are split so the first add (and with it
                    # the whole scalar stream) starts ~4us earlier.
                    half = COLS // 2
                    nc.sync.dma_start(xt[:, :half], x_d[rows, :half])
                    nc.sync.dma_start(nt[:, :half], n_d[rows, :half])
                    nc.sync.dma_start(xt[:, half:], x_d[rows, half:])
                    nc.sync.dma_start(nt[:, half:], n_d[rows, half:])
                else:
                    nc.sync.dma_start(xt[:], x_d[rows, :])
                    nc.sync.dma_start(nt[:], n_d[rows, :])
                xts.append(xt)
                nts.append(nt)

            yts = []
            for t in range(NT):
                rows = slice(t * 128, (t + 1) * 128)
                yt = iopool.tile([128, COLS], f32, tag=f"yt{t}")
                if t == 0:
                    half = COLS // 2
                    nc.vector.tensor_tensor(yt[:, :half], xts[t][:, :half],
                                            nts[t][:, :half], op=op_add)
                    nc.vector.tensor_tensor(yt[:, half:], xts[t][:, half:],
                                            nts[t][:, half:], op=op_add)
                else:
                    nc.vector.tensor_tensor(yt[:], xts[t][:], nts[t][:],
                                            op=op_add)
                yts.append(yt)
                # Finer chunks on the last tile keep its compute+store tail
                # short; the rings are already saturated for earlier tiles.
                ch = CH // 2 if t == NT - 1 else CH
                for c in range(COLS // ch):
                    cols = slice(c * ch, (c + 1) * ch)
                    pt = tpool.tile([128, ch], f32, tag=f"pt{t % 2}")
                    nc.scalar.activation(pt[:], yt[:, cols], Tanh,
                                         bias=bp[:, t:t + 1], scale=sc[:, t:t + 1])
                    qt = tpool.tile([128, ch], f32, tag=f"qt{t % 2}")
                    nc.scalar.activation(qt[:], yt[:, cols], Tanh,
                                         bias=bq[:, t:t + 1], scale=sc[:, t:t + 1])

                    nc.vector.tensor_tensor(pt[:], pt[:], qt[:], op=op_sub)
                    nc.vector.tensor_scalar(pt[:], pt[:], 0.5, 1e-9,
                                            op0=op_mult, op1=op_max)
                    nc.scalar.dma_start(l_d[rows, cols], pt[:])

            # y stores ride the sync FIFO BEHIND its loads: by the time the
            # FIFO drains the loads, every y tile is already computed, so the
            # final ring work is never compute-gated.
            for t in range(NT):
                rows = slice(t * 128, (t + 1) * 128)
                nc.sync.dma_start(y_d[rows, :], yts[t][:])

    nc.compile()
    return nc


def _build_program_raw():
    """Hand-scheduled variant: explicit per-engine instruction streams and
    semaphores instead of the Tile scheduler.

    sync   : param + x/noise loads (HWDGE FIFO), then y stores
    scalar : tanh pairs per 1024-col chunk, lik store issues (ACT FIFO)
    vector : adds (whole tile), sub + scale/clamp per chunk
    """
    import concourse.bacc as bacc
    import concourse.mybir as mybir

    f32 = mybir.dt.float32
    nc = bacc.Bacc("TRN2", target_bir_lowering=False, debug=False,
                   num_devices=NCORES)

    x_d = nc.dram_tensor("x", [ROWS, COLS], f32, kind="ExternalInput")
    n_d = nc.dram_tensor("noise", [ROWS, COLS], f32, kind="ExternalInput")
    sc_d = nc.dram_tensor("scl", [128, NT], f32, kind="ExternalInput")
    bp_d = nc.dram_tensor("bp", [128, NT], f32, kind="ExternalInput")
    bq_d = nc.dram_tensor("bq", [128, NT], f32, kind="ExternalInput")
    y_d = nc.dram_tensor("y", [ROWS, COLS], f32, kind="ExternalOutput")
    l_d = nc.dram_tensor("lik", [ROWS, COLS], f32, kind="ExternalOutput")

    Tanh = mybir.ActivationFunctionType.Tanh
    op_add = mybir.AluOpType.add
    op_sub = mybir.AluOpType.subtract
    op_mult = mybir.AluOpType.mult
    op_max = mybir.AluOpType.max

    CH = 1024
    NCH = COLS // CH

    sct = nc.alloc_sbuf_tensor("sct", [128, NT], f32)
    bpt = nc.alloc_sbuf_tensor("bpt", [128, NT], f32)
    bqt = nc.alloc_sbuf_tensor("bqt", [128, NT], f32)
    xts = [nc.alloc_sbuf_tensor(f"xt{t}", [128, COLS], f32) for t in range(NT)]
    nts = [nc.alloc_sbuf_tensor(f"nt{t}", [128, COLS], f32) for t in range(NT)]
    yts = [nc.alloc_sbuf_tensor(f"yt{t}", [128, COLS], f32) for t in range(NT)]
    pts = [nc.alloc_sbuf_tensor(f"pt{i}", [128, CH], f32) for i in range(NT * NCH)]
    qts = [nc.alloc_sbuf_tensor(f"qt{i}", [128, CH], f32) for i in range(NT * NCH)]

    ld = nc.alloc_semaphore("ld")    # load completions (+16 each)
    va = nc.alloc_semaphore("va")    # vector adds (+1 each)
    sa = nc.alloc_semaphore("sa")    # scalar acts (+1 each)
    vt = nc.alloc_semaphore("vt")    # vector sub+ts chains (+1 per chunk)
    ys = nc.alloc_semaphore("ys")    # y store completions
    ls = nc.alloc_semaphore("ls")    # lik store completions

    with nc.Block() as block:

        @block.sync
        def _(sync):
            sync.dma_start(sct[:], sc_d[:]).then_inc(ld, 16)
            sync.dma_start(bpt[:], bp_d[:]).then_inc(ld, 16)
            sync.dma_start(bqt[:], bq_d[:]).then_inc(ld, 16)
            for t in range(NT):
                rows = slice(t * 128, (t + 1) * 128)
                sync.dma_start(xts[t][:], x_d[rows, :]).then_inc(ld, 16)
                sync.dma_start(nts[t][:], n_d[rows, :]).then_inc(ld, 16)
            for t in range(NT):
                rows = slice(t * 128, (t + 1) * 128)
                sync.wait_ge(va, t + 1)
                sync.dma_start(y_d[rows, :], yts[t][:]).then_inc(ys, 16)
            sync.wait_ge(ys, NT * 16)

        @block.vector
        def _(vector):
            # adds as soon as each tile's pair lands; sub/ts chunks slot in
            # between, ordered to never stall ahead of a satisfiable wait.
            def sub_ts(t, c):
                i = t * NCH + c
                vector.wait_ge(sa, 2 * (i + 1))
                nc.vector.tensor_tensor(pts[i][:], pts[i][:], qts[i][:],
                                        op=op_sub)
                nc.vector.tensor_scalar(pts[i][:], pts[i][:], 0.5, 1e-9,
                                        op0=op_mult, op1=op_max).then_inc(vt, 1)

            vector.wait_ge(ld, (3 + 2) * 16)  # params + x0,n0
            nc.vector.tensor_tensor(yts[0][:], xts[0][:], nts[0][:],
                                    op=op_add).then_inc(va, 1)
            vector.wait_ge(ld, (3 + 4) * 16)  # + x1,n1
            nc.vector.tensor_tensor(yts[1][:], xts[1][:], nts[1][:],
                                    op=op_add).then_inc(va, 1)
            sub_ts(0, 0)
            sub_ts(0, 1)
            vector.wait_ge(ld, (3 + 6) * 16)  # + x2,n2
            nc.vector.tensor_tensor(yts[2][:], xts[2][:], nts[2][:],
                                    op=op_add).then_inc(va, 1)
            sub_ts(1, 0)
            sub_ts(1, 1)
            sub_ts(2, 0)
            sub_ts(2, 1)

        @block.scalar
        def _(scalar):
            def acts(t, c):
                i = t * NCH + c
                cols = slice(c * CH, (c + 1) * CH)
                nc.scalar.activation(pts[i][:], yts[t][:, cols], Tanh,
                                     bias=bpt[:, t:t + 1],
                                     scale=sct[:, t:t + 1]).then_inc(sa, 1)
                nc.scalar.activation(qts[i][:], yts[t][:, cols], Tanh,
                                     bias=bqt[:, t:t + 1],
                                     scale=sct[:, t:t + 1]).then_inc(sa, 1)

            def store(t, c):
                i = t * NCH + c
                cols = slice(c * CH, (c + 1) * CH)
                rows = slice(t * 128, (t + 1) * 128)
                scalar.wait_ge(vt, i + 1)
                scalar.dma_start(l_d[rows, cols], pts[i][:]).then_inc(ls, 16)

            scalar.wait_ge(va, 1)
            acts(0, 0)
            acts(0, 1)
            scalar.wait_ge(va, 2)
            acts(1, 0)
            store(0, 0)
            acts(1, 1)
            store(0, 1)
            scalar.wait_ge(va, 3)
            acts(2, 0)
            store(1, 0)
            acts(2, 1)
            store(1, 1)
            store(2, 0)
            store(2, 1)
            scalar.wait_ge(ls, NT * NCH * 16)

    nc.compile()
    return nc


def _get_program():
    if "nc" not in _CACHE:
        import os

        raw = os.environ.get("EB_RAW", "1") == "1"
        _CACHE["nc"] = _build_program_raw() if raw else _build_program()
    return _CACHE["nc"]


def kernel(x, noise, w0, b0, f0, w1, b1, f1, w2, b2, f2, w3, b3):
    from concourse.bass_utils import run_bass_kernel_spmd

    ws = [w0, w1, w2, w3]
    bs = [b0, b1, b2, b3]
    fs = [f0, f1, f2]

    if any(np.any(np.asarray(f) != 0.0) for f in fs):
        # Gated (non-affine) case: bit-accurate host fallback. Never taken for
        # this module's initialization (all gates are zero).
        return _numpy_fallback(x, noise, ws, bs, fs)

    M, D = _fold_affine(ws, bs)  # (C,) float64 each, M > 0
    ch = np.arange(ROWS) // 2  # channel id per folded row
    Mr, Dr = M[ch], D[ch]
    # p/q = tanh(M/2 * y + (D +- M/2)/2); lik = max(0.5*(p - q), 1e-9)
    scl = (Mr / 2).astype(np.float32).reshape(NT, 128).T.copy()
    bpv = (Dr / 2 + Mr / 4).astype(np.float32).reshape(NT, 128).T.copy()
    bqv = (Dr / 2 - Mr / 4).astype(np.float32).reshape(NT, 128).T.copy()

    x = np.ascontiguousarray(np.asarray(x, np.float32))
    noise = np.ascontiguousarray(np.asarray(noise, np.float32))

    nc = _get_program()
    in_maps = [
        {
            "x": x[b].reshape(ROWS, COLS),
            "noise": noise[b].reshape(ROWS, COLS),
            "scl": scl,
            "bp": bpv,
            "bq": bqv,
        }
        for b in range(NCORES)
    ]
    res = run_bass_kernel_spmd(nc, in_maps, list(range(NCORES))).results

    y = np.stack([res[b]["y"].reshape(C, H, W) for b in range(NCORES)])
    lik = np.stack([res[b]["lik"].reshape(C, H, W) for b in range(NCORES)])
    return y, lik
